# revision 4
# baseline (speedup 1.0000x reference)
"""AliasFreeActivation (upsample2x -> leaky_relu -> 31x31 depthwise sinc conv
-> downsample2x) as a Trainium2 Bass/Tile kernel, data-parallel over 8 cores.

Math (per [128,128] image; B*C = 512 images, 64 per core):
  out = Dy @ Conv_F(lrelu(Uy @ x @ Ux^T)) @ Dx^T
With F = sum_r g_r h_r^T (SVD of the 31x31 filter, effective rank 11):
  out = sum_r M_r @ act @ N_r^T
  M_r = Dy @ Toeplitz(g_r) [128,256],  N_r = Dx @ Toeplitz(h_r) [128,256]
  act = lrelu(Uy @ x @ Ux^T) [256,256]
All heavy work is dense matmuls on TensorE; downsample is folded into M/N.

Device dataflow per image (out[m,n] = sum_k lhsT[k,m] rhs[k,n]):
  S1a: tmpT[c,Y]    = sum_y x[y,c] UyT[y,Y]
  S1b: actT[X,Y]    = sum_c UxT[c,X] tmpT[c,Y]   (+ lrelu on evac)
  A:   W[Y,(r,j)]   = sum_X actT[X,Y] NT[X,(r,j)]
  B:   out[i,(m,j)] = sum_{r,Yc} MT[Yc,(r,i)] W[Yc,(r,m,j)]   (PSUM accum)
"""
import contextlib

import numpy as np

import concourse.bass as bass
import concourse.mybir as mybir
import concourse.tile as tile
from concourse import bacc
from concourse.bass_utils import run_bass_kernel_spmd

H = 128
H2 = 256
KF = 31
LRELU_SLOPE = 0.01
RANK = 11
GROUP = 4
N_CORES = 8
N_IMG = 64                      # images per core (512 / 8)
DT_MM = mybir.dt.float32        # matmul operand dtype (float32 | float32r)


# ---------------- host-side constants ----------------

def _ac_matrix(out_n, in_n):
    scale = (in_n - 1) / (out_n - 1)
    c = np.arange(out_n, dtype=np.float64) * scale
    i0 = np.clip(np.floor(c).astype(np.int64), 0, in_n - 2)
    w = c - i0
    M = np.zeros((out_n, in_n), dtype=np.float64)
    M[np.arange(out_n), i0] = 1.0 - w
    M[np.arange(out_n), i0 + 1] = w
    return M


def _toeplitz_same(h, n):
    T = np.zeros((n, n), dtype=np.float64)
    for u in range(len(h)):
        d = u - len(h) // 2
        if d >= 0:
            idx = np.arange(0, n - d)
        else:
            idx = np.arange(-d, n)
        T[idx, idx + d] += h[u]
    return T


def _make_consts(filt, rank):
    F = np.asarray(filt, dtype=np.float64)
    U, S, Vt = np.linalg.svd(F)
    D = _ac_matrix(H, H2)
    Uu = _ac_matrix(H2, H)
    uyt = np.ascontiguousarray(Uu.T).astype(np.float32)     # [128 y, 256 Y]
    nt = np.zeros((2, H, rank * H), dtype=np.float32)
    mt = np.zeros((2, H, rank * H), dtype=np.float32)
    for r in range(rank):
        g = U[:, r] * np.sqrt(S[r])
        h = Vt[r, :] * np.sqrt(S[r])
        Mr = D @ _toeplitz_same(g, H2)
        Nr = D @ _toeplitz_same(h, H2)
        for c in range(2):
            nt[c, :, r * H:(r + 1) * H] = Nr[:, c * H:(c + 1) * H].T.astype(np.float32)
            mt[c, :, r * H:(r + 1) * H] = Mr[:, c * H:(c + 1) * H].T.astype(np.float32)
    return {"uyt": uyt, "uxt": uyt.copy(), "nt": nt, "mt": mt}


# ---------------- device program ----------------

def _build_tile_program(tc, outs, ins, *, n_img, rank, group, dt_mm):
    nc = tc.nc
    x_d, uyt_d, uxt_d, nt_d, mt_d = ins
    out_d = outs[0]
    RC = rank * H
    G = group
    GW = G * H
    assert n_img % G == 0
    f32 = mybir.dt.float32

    segs = []
    s = 0
    while s < RC:
        segs.append((s, min(s + 512, RC)))
        s = min(s + 512, RC)

    ctx = contextlib.ExitStack()
    with ctx:
        const_pool = ctx.enter_context(tc.tile_pool(name="consts", bufs=1))
        x_pool = ctx.enter_context(tc.tile_pool(name="x", bufs=2))
        tmp_pool = ctx.enter_context(tc.tile_pool(name="tmp", bufs=2))
        act_pool = ctx.enter_context(tc.tile_pool(name="act", bufs=2))
        w_pool = ctx.enter_context(tc.tile_pool(name="w", bufs=2))
        osb_pool = ctx.enter_context(tc.tile_pool(name="osb", bufs=2))
        ps_small = ctx.enter_context(tc.tile_pool(name="ps_small", bufs=2, space="PSUM"))
        ps_w = ctx.enter_context(tc.tile_pool(name="ps_w", bufs=4, space="PSUM"))
        ps_out = ctx.enter_context(tc.tile_pool(name="ps_out", bufs=1, space="PSUM"))

        uyt_sb = const_pool.tile([H, H2], dt_mm, tag="uyt")
        nc.sync.dma_start(uyt_sb[:], uyt_d[:])
        uxt_sb = const_pool.tile([H, H2], dt_mm, tag="uxt")
        nc.sync.dma_start(uxt_sb[:], uxt_d[:])
        nt_sb = []
        mt_sb = []
        for c in range(2):
            t = const_pool.tile([H, RC], dt_mm, tag=f"nt{c}", name=f"nt{c}_sb")
            nc.sync.dma_start(t[:], nt_d[c])
            nt_sb.append(t)
            t = const_pool.tile([H, RC], dt_mm, tag=f"mt{c}", name=f"mt{c}_sb")
            nc.sync.dma_start(t[:], mt_d[c])
            mt_sb.append(t)

        for g in range(n_img // G):
            x_sb = x_pool.tile([H, GW], dt_mm, tag="x")
            xg = x_d[g * G:(g + 1) * G].rearrange("g h w -> h g w")
            nc.sync.dma_start(x_sb[:].rearrange("h (g w) -> h g w", g=G), xg)

            wg_sb = [w_pool.tile([H, rank * GW], dt_mm, tag=f"wg{yc}",
                                 name=f"wg{yc}_{g}") for yc in range(2)]

            for m in range(G):
                # S1a
                tmpT_ps = ps_small.tile([H, H2], f32, tag="sm")
                nc.tensor.matmul(tmpT_ps[:], x_sb[:, m * H:(m + 1) * H], uyt_sb[:],
                                 start=True, stop=True)
                tmpT_sb = tmp_pool.tile([H, H2], dt_mm, tag="tmpT")
                nc.vector.tensor_copy(tmpT_sb[:], tmpT_ps[:])

                # S1b + lrelu
                act_ps = ps_small.tile([H, 2 * H2], f32, tag="sm")
                for xc in range(2):
                    nc.tensor.matmul(act_ps[:, xc * H2:(xc + 1) * H2],
                                     uxt_sb[:, xc * H:(xc + 1) * H], tmpT_sb[:],
                                     start=True, stop=True)
                act_sb = act_pool.tile([H, 2 * H2], dt_mm, tag="act")
                for xc in range(2):
                    nc.scalar.activation(act_sb[:, xc * H2:(xc + 1) * H2],
                                         act_ps[:, xc * H2:(xc + 1) * H2],
                                         mybir.ActivationFunctionType.Lrelu,
                                         alpha=LRELU_SLOPE)

                # pass A
                for yc in range(2):
                    for si, (s0, s1) in enumerate(segs):
                        w_ps = ps_w.tile([H, 512], f32, tag="wps", name=f"wps_{g}_{m}_{yc}_{si}")
                        for xc in range(2):
                            nc.tensor.matmul(
                                w_ps[:, 0:s1 - s0],
                                act_sb[:, xc * H2 + yc * H: xc * H2 + (yc + 1) * H],
                                nt_sb[xc][:, s0:s1],
                                start=(xc == 0), stop=(xc == 1))
                        nr = (s1 - s0) // H
                        r0 = s0 // H
                        src = w_ps[:, 0:s1 - s0].rearrange("p (r j) -> p r j", r=nr)
                        full = wg_sb[yc][:].rearrange("p (r g w) -> p r g w", r=rank, g=G)
                        dst = full[:, r0:r0 + nr, m]
                        if si % 3 == 2:
                            nc.scalar.activation(dst, src,
                                                 mybir.ActivationFunctionType.Copy)
                        else:
                            nc.vector.tensor_copy(dst, src)

            # pass B
            out_ps = ps_out.tile([H, GW], f32, tag="ops", name=f"ops_{g}")
            nmm = 0
            for yc in range(2):
                for r in range(rank):
                    nmm += 1
                    nc.tensor.matmul(
                        out_ps[:],
                        mt_sb[yc][:, r * H:(r + 1) * H],
                        wg_sb[yc][:, r * GW:(r + 1) * GW],
                        start=(nmm == 1), stop=(nmm == 2 * rank))
            out_sb = osb_pool.tile([H, GW], f32, tag="osb")
            nc.scalar.activation(out_sb[:], out_ps[:],
                                 mybir.ActivationFunctionType.Copy)
            og = out_d[g * G:(g + 1) * G].rearrange("g h w -> h g w")
            nc.sync.dma_start(og, out_sb[:].rearrange("h (g w) -> h g w", g=G))


_NC_CACHE = {}


def _build_nc(n_img=N_IMG, rank=RANK, group=GROUP, dt_mm=DT_MM):
    key = (n_img, rank, group, dt_mm)
    if key in _NC_CACHE:
        return _NC_CACHE[key]
    nc = bacc.Bacc("TRN2", target_bir_lowering=False, debug=False)
    f32 = mybir.dt.float32
    x_d = nc.dram_tensor("x", [n_img, H, H], dt_mm, kind="ExternalInput").ap()
    uyt_d = nc.dram_tensor("uyt", [H, H2], dt_mm, kind="ExternalInput").ap()
    uxt_d = nc.dram_tensor("uxt", [H, H2], dt_mm, kind="ExternalInput").ap()
    nt_d = nc.dram_tensor("nt", [2, H, rank * H], dt_mm, kind="ExternalInput").ap()
    mt_d = nc.dram_tensor("mt", [2, H, rank * H], dt_mm, kind="ExternalInput").ap()
    out_d = nc.dram_tensor("out", [n_img, H, H], f32, kind="ExternalOutput").ap()
    with tile.TileContext(nc) as tc:
        _build_tile_program(tc, [out_d], [x_d, uyt_d, uxt_d, nt_d, mt_d],
                            n_img=n_img, rank=rank, group=group, dt_mm=dt_mm)
    nc.compile()
    _NC_CACHE[key] = nc
    return nc


def run(x, filt, trace=False, trace_kwargs=None):
    """Run on 8 cores. Returns (out [B,C,H,W] f32, exec_time_ns or None)."""
    x = np.ascontiguousarray(np.asarray(x, dtype=np.float32))
    filt = np.asarray(filt, dtype=np.float32)
    B, C, Hh, Ww = x.shape
    assert (Hh, Ww) == (H, H) and B * C == N_CORES * N_IMG
    consts = _make_consts(filt, RANK)
    nc = _build_nc()
    imgs = x.reshape(N_CORES, N_IMG, H, H)
    in_maps = []
    for core in range(N_CORES):
        in_maps.append({
            "x": np.ascontiguousarray(imgs[core]),
            "uyt": consts["uyt"], "uxt": consts["uxt"],
            "nt": consts["nt"], "mt": consts["mt"],
        })
    res = run_bass_kernel_spmd(nc, in_maps, core_ids=list(range(N_CORES)),
                               trace=trace, **(trace_kwargs or {}))
    out = np.stack([res.results[c]["out"] for c in range(N_CORES)])
    return out.reshape(B, C, H, H).astype(np.float32, copy=False), res.exec_time_ns


def kernel(x, filt):
    return run(x, filt, trace=False)[0]
